# revision 78
# baseline (speedup 1.0000x reference)
"""Trainium2 Bass kernel for a transformer encoder layer (nn_Encoder).

x:[2,2048,1024] f32. 8 NeuronCores, data-parallel: core c handles batch
n=c//4, query rows qi=c%4 (512 tokens). K/V are recomputed per core for
the full batch (x4 redundancy) to avoid collectives, which are far too
slow (~300us for the 8.4MB all-reduce this would replace).

All matmul operands are bf16 (inputs quantized on host): same PE rate as
float32r at 512-wide tiles, but full rate at narrow tiles too, which lets
the PV (attn @ V) product run as [q,65]-output matmuls at half the PE rows
of the [hd,q] formulation. Softmax divide is then a per-partition scalar
multiply, and a cheap bf16 PE transpose restores [hd,q] for the output
projection. bf16 also halves all weight DMA traffic.
"""
import os
import sys
from contextlib import ExitStack

for _p in ("/opt/trn_rl_repo", "/root/.axon_site/_ro/trn_rl_repo"):
    if os.path.isdir(_p) and _p not in sys.path:
        sys.path.insert(0, _p)

import numpy as np
import ml_dtypes
import concourse.bass as bass
import concourse.mybir as mybir
import concourse.tile as tile
from concourse import bacc
from concourse.bass_utils import run_bass_kernel_spmd
from concourse.masks import make_identity

F32 = mybir.dt.float32
BF16 = mybir.dt.bfloat16
AF = mybir.ActivationFunctionType
ALU = mybir.AluOpType
BF = ml_dtypes.bfloat16

D = 1024
H = 16
HD = 64
FF = 4096
L = 2048
NB = 2
P = 128
QT = 512          # query tokens per core
DC = D // P       # 8 chunks of the model dim
KT = L // P       # 16 key tiles
FC = FF // P      # 32 ff chunks
TT = QT // P      # 4 own-token tiles
NPAIR = H // 2    # 8 head pairs
EPS = 1e-5

_CACHED_NC = {}


def _build_nc(affine=True):
    nc = bacc.Bacc("TRN2", target_bir_lowering=False)

    din = {}

    def dparam(name, shape, dt=BF16):
        din[name] = nc.dram_tensor(name, shape, dt, kind="ExternalInput")
        return din[name]

    # x[n].T with tokens permuted so this core's own 512 come first (softmax
    # over keys is order-invariant, so the permutation only matters for Q)
    xT = dparam("xT", [D, L])
    xq = dparam("xq", [QT, D], F32)        # own tokens, natural (residual)
    wq = dparam("wq", [NPAIR, DC, P, P])   # [pair, dc, dpart, cols]
    wk = dparam("wk", [NPAIR, DC, P, P])
    wv = dparam("wv", [2, DC, P, D // 2])  # [vcol-half, dc, dpart, 512]
    wo = dparam("wo", [DC, P, D])          # [hd-chunk, hd-part, ocols]
    w1 = dparam("w1", [FC, DC, P, P])      # [fc, dc, dpart, fcols]
    w2 = dparam("w2", [FC, P, D])          # [fc, ff-part, ocols]
    bq = dparam("bq", [P, NPAIR], F32)     # per-partition bias, by pair
    bk = dparam("bk", [P, NPAIR], F32)
    b1 = dparam("b1", [P, FC], F32)
    bvb = dparam("bvb", [P, D], F32)       # host-broadcast per-column params
    b2b = dparam("b2b", [P, D], F32)
    g1b = dparam("g1b", [P, D], F32)
    be1b = dparam("be1b", [P, D], F32)
    g2b = dparam("g2b", [P, D], F32)
    be2b = dparam("be2b", [P, D], F32)
    vones = dparam("vones", [P, KT], BF16)

    y = nc.dram_tensor("y", [QT, D], F32, kind="ExternalOutput")

    with tile.TileContext(nc) as tc:
        with tc.tile_pool(name="pers", bufs=1) as pers:
            # ---- persistent blobs (tag-shared slots across phases) ----
            # blobA: xT (proj) -> ff1 (ffn);  32KB/partition each
            # tok1:  aqAll (attn out, [q,hd]) -> hT (ffn)
            # tok2:  qT (proj+attn) -> h (post-LN1, natural, bf16)
            # tok3:  outSB (attn out transposed, [hd,q])
            bq_t = pers.tile([P, NPAIR], F32, tag="bq")
            # weights for the first Q-proj pair go first so PE can start
            # ~4us in; the own-token xT columns stream right behind them.
            qT_t = pers.tile([P, NPAIR, QT], BF16, tag="tok2")
            # pools that outlive several phases (closed via _pre at the end);
            # opened before the Q block so V/attention weight prefetches can
            # be emitted from inside it
            _pre = ExitStack()
            kp = _pre.enter_context(tc.tile_pool(name="kp", bufs=1))
            op = _pre.enter_context(tc.tile_pool(name="op", bufs=1))
            lnw = _pre.enter_context(tc.tile_pool(name="lnw", bufs=1))
            fp = _pre.enter_context(tc.tile_pool(name="fp", bufs=5))
            vpw = _pre.enter_context(tc.tile_pool(name="vpw", bufs=2))
            xT_t = pers.tile([P, DC, L], BF16, tag="blobA")
            with tc.tile_pool(name="qp", bufs=4) as qp, \
                 tc.tile_pool(name="qps", bufs=4, space="PSUM") as qps:
                wq_ts = {}

                def load_wq(pr):
                    wq_t = qp.tile([P, DC, P], BF16, tag="w",
                                   name=f"wq{pr}")
                    nc.scalar.dma_start(
                        wq_t[:], wq[pr].rearrange("c p m -> p c m"))
                    wq_ts[pr] = wq_t

                wv_ts = [vpw.tile([P, DC, 512], BF16, tag="wvh",
                                  name=f"wvh{vh}") for vh in range(2)]
                load_wq(0)
                xT_r = xT.rearrange("(c p) t -> p c t", p=P)
                # own-token columns (the Q-proj moving operand) land first
                nc.sync.dma_start(xT_t[:, 0:4, 0:QT], xT_r[:, 0:4, 0:QT])
                nc.sync.dma_start(xT_t[:, 4:8, 0:QT], xT_r[:, 4:8, 0:QT])
                nc.scalar.dma_start(bq_t[:], bq[:])
                load_wq(1)
                load_wq(2)
                load_wq(3)
                for pr in range(NPAIR):
                    wq_t = wq_ts[pr]
                    ps = qps.tile([P, 512], F32, tag="mm")
                    for dc in range(DC):
                        nc.tensor.matmul(ps[:], wq_t[:, dc, :],
                                         xT_t[:, dc, 0:QT],
                                         start=(dc == 0), stop=(dc == DC - 1))
                    nc.vector.tensor_scalar(qT_t[:, pr, :], ps[:],
                                            bq_t[:, pr:pr + 1], None, ALU.add)
                    if pr + 4 < NPAIR:
                        load_wq(pr + 4)
                    # bulk loads for V ride the scalar queue between the wq
                    # chunks, ordered by first use: xT tail < wv0 < wv1
                    if pr == 0:
                        nc.scalar.dma_start(xT_t[:, 5:8, QT:L],
                                            xT_r[:, 5:8, QT:L])
                    if pr == 1:
                        nc.scalar.dma_start(
                            wv_ts[0][:], wv[0].rearrange("c p m -> p c m"))
                    if pr == 3:
                        nc.scalar.dma_start(
                            wv_ts[1][:], wv[1].rearrange("c p m -> p c m"))

            ident = pers.tile([P, P], BF16, tag="ident")
            make_identity(nc, ident[:])
            bk_t = pers.tile([P, NPAIR], F32, tag="bk")
            b1_t = pers.tile([P, FC], F32, tag="b1")
            eps_t = pers.tile([P, 1], F32, tag="eps")
            nc.gpsimd.memset(eps_t[:], EPS)
            zero_t = pers.tile([P, 1], F32, tag="zero")
            nc.gpsimd.memset(zero_t[:], 0.0)

            # Whole wk preloaded in one gated DMA (2MB) right after the
            # startup stream drains: attention then needs NO timely DMA at
            # all, so the (serialized) DMA device is free for the epoch2
            # prefetch flood without stalling the per-pair K projections.
            wk_all = kp.tile([P, NPAIR, DC, P], BF16, tag="wall")

            # Bulk prefetches for out-proj / FFN weights run on the (idle)
            # gpsimd queue, but the Tile scheduler reorders by data deps, so
            # each prefetch tile first gets a tiny gpsimd write sourced from
            # the last Q-proj output: the WAW dependency keeps these loads
            # out of the startup window where wq/xTq need the full DMA
            # bandwidth — yet they still land ~200us before use.
            # Epoch marker: written once when the last Q bias-add lands
            # (~18us, end of the startup DMA crunch). Gates read the epoch —
            # NOT qT_t directly, whose SBUF slot is later reused by h_t and
            # would create spurious (even cyclic) dependencies.
            epoch = pers.tile([P, 8], F32, tag="epoch")
            nc.gpsimd.tensor_copy(epoch[0:1, :], qT_t[0:1, NPAIR - 1, 0:8])
            # epoch2 fires once V's last bias-add lands (~77us): DMA is a
            # single shared device here, so the bulk FFN/out-proj prefetch
            # flood must wait until the whole startup stream (wq/xTq/xT/wv)
            # has drained, not just Q-proj.
            epoch2 = pers.tile([P, 8], F32, tag="epoch2")

            def gate(dst_slice, width, ep=None):
                nc.gpsimd.tensor_copy(dst_slice,
                                      (ep or epoch)[0:1, 0:width])

            gate(wk_all[0:1, :, 0, 0], NPAIR)
            nc.gpsimd.dma_start(wk_all[:],
                                wk.rearrange("r c p m -> p r c m"))

            w1_ts = {}

            def load_w1(fc4, engs):
                w1_t = fp.tile([P, 4, DC, P], BF16, tag="wstream",
                               name=f"w1_{fc4}")
                gate(w1_t[0:1, :, 0, 0], 4, epoch2)
                engs[0].dma_start(
                    w1_t[:, 0:2, :, :],
                    w1[4 * fc4:4 * fc4 + 2].rearrange("f c p m -> p f c m"))
                engs[1].dma_start(
                    w1_t[:, 2:4, :, :],
                    w1[4 * fc4 + 2:4 * fc4 + 4].rearrange("f c p m -> p f c m"))
                w1_ts[fc4] = w1_t

            _att_es = ExitStack()
            vap = _att_es.enter_context(tc.tile_pool(name="vap", bufs=1))
            v_aug = vap.tile([P, KT, H * (HD + 1)], BF16, tag="vaug")
            ones_t = pers.tile([P, KT], BF16, tag="ones")
            nc.scalar.dma_start(bk_t[:], bk[:])
            nc.scalar.dma_start(b1_t[:], b1[:])

            nc.sync.dma_start(xT_t[:, 0:3, QT:L], xT_r[:, 0:3, QT:L])
            nc.sync.dma_start(xT_t[:, 3:5, QT:L], xT_r[:, 3:5, QT:L])
            nc.scalar.dma_start(ones_t[:], vones[:])
            nc.vector.tensor_copy(
                v_aug[:].rearrange("p t (h c) -> p t h c", c=HD + 1)[:, :, :, HD],
                ones_t[:, :, None].to_broadcast([P, KT, H]))

            # ================= V projection (dc-outer, streamed wv) ===
            with tc.tile_pool(name="vp", bufs=1) as vp, \
                 tc.tile_pool(name="vps", bufs=1, space="PSUM") as vps:
                bvb_t = vp.tile([P, D], F32, tag="bvb")
                nc.scalar.dma_start(bvb_t[:], bvb[:])
                for vh in range(2):
                    wv_t = wv_ts[vh]
                    for pas in range(4):
                        # alternate bank sets per group so group g+1's
                        # matmuls don't wait on group g's DVE drains
                        par = (vh * 4 + pas + 1) % 2
                        ps4 = [vps.tile([P, 512], F32, tag=f"vmm{par}_{i}",
                                        name=f"vps_{vh}_{pas}_{i}") for i in range(4)]
                        for dc in range(DC):
                            for i in range(4):
                                tt = pas * 4 + i
                                nc.tensor.matmul(
                                    ps4[i], xT_t[:, dc, tt * P:(tt + 1) * P],
                                    wv_t[:, dc, :], start=(dc == 0), stop=(dc == DC - 1))
                        for i in range(4):
                            tt = pas * 4 + i
                            dst = v_aug[:, tt, :].rearrange(
                                "p (h c) -> p h c", c=HD + 1)[:, vh * 8:(vh + 1) * 8, 0:HD]
                            nc.vector.tensor_tensor(
                                dst, ps4[i].rearrange("p (h c) -> p h c", c=HD),
                                bvb_t[:, vh * 512:(vh + 1) * 512].rearrange(
                                    "p (h c) -> p h c", c=HD),
                                ALU.add)

            # epoch2: the last V bias-add region — startup DMA has fully
            # drained once this lands, so the bulk prefetch flood may start.
            nc.gpsimd.tensor_copy(
                epoch2[0:1, :],
                v_aug[0:1, KT - 1, (H - 1) * (HD + 1):(H - 1) * (HD + 1) + 8])
            wo_t = op.tile([P, DC, D], BF16, tag="wof")
            gate(wo_t[0:1, :, 0], DC, epoch2)
            wo_r = wo.rearrange("c p m -> p c m")
            nc.gpsimd.dma_start(wo_t[:, 0:4, :], wo_r[:, 0:4, :])
            nc.gpsimd.dma_start(wo_t[:, 4:8, :], wo_r[:, 4:8, :])
            xq_s = op.tile([P, TT, D], F32, tag="xqs")
            gate(xq_s[0:1, :, 0], TT, epoch2)
            nc.gpsimd.dma_start(xq_s[:], xq.rearrange("(t p) d -> p t d", p=P))
            if affine:
                g1b_t = lnw.tile([P, D], F32, tag="g1b")
                be1b_t = lnw.tile([P, D], F32, tag="be1b")
                gate(g1b_t[0:1, 0:1], 1, epoch2)
                gate(be1b_t[0:1, 0:1], 1, epoch2)
                nc.gpsimd.dma_start(g1b_t[:], g1b[:])
                nc.gpsimd.dma_start(be1b_t[:], be1b[:])
            else:
                g1b_t = be1b_t = None
            load_w1(0, (nc.gpsimd, nc.gpsimd))
            load_w1(1, (nc.gpsimd, nc.gpsimd))

            # ========== K projection fused with attention, per pair ====
            # kT for a pair stays in SBUF (no DRAM bounce); PSUM budget:
            # K accum 2 + S 2x2 + PV accum 2 = 8 banks exactly.
            # PV runs as [q,65]-output matmuls (65 moving rows, bf16 full
            # rate): stationary = exp(S) chunk [128k x 128q], moving =
            # v_aug slice [128k x 65]; accumulated over the 16 key tiles.
            # Softmax denominator rides along as v_aug column 64 (ones),
            # so the divide is a per-partition reciprocal+scale on DVE.
            aqAll = pers.tile([P, NPAIR, TT, P], BF16, tag="tok1")
            with tc.tile_pool(name="kq", bufs=2) as kq, \
                 tc.tile_pool(name="atp", bufs=3) as atp, \
                 tc.tile_pool(name="atd", bufs=2) as atd, \
                 tc.tile_pool(name="kps", bufs=1, space="PSUM") as kps, \
                 tc.tile_pool(name="stp", bufs=2, space="PSUM") as stpool, \
                 tc.tile_pool(name="pvp", bufs=1, space="PSUM") as pvpool:
                for pr in range(NPAIR):
                    wk_t = wk_all[:, pr, :, :]
                    kT_sb = kq.tile([P, L], BF16, tag="kts", name=f"kts_{pr}")
                    # 4 quarters alternating 2 psum banks: quarter q+1's
                    # matmuls overlap quarter q's bias-add drain
                    for q4 in range(4):
                        pst = kps.tile([P, 512], F32, tag=f"kmm{q4 % 2}",
                                       name=f"kps_{pr}_{q4}")
                        for dc in range(DC):
                            nc.tensor.matmul(
                                pst[:], wk_t[:, dc, :],
                                xT_t[:, dc, q4 * 512:(q4 + 1) * 512],
                                start=(dc == 0), stop=(dc == DC - 1))
                        nc.vector.tensor_scalar(
                            kT_sb[:, q4 * 512:(q4 + 1) * 512], pst[:],
                            bk_t[:, pr:pr + 1], None, ALU.add)
                    # ---- attention for this pair (kT_sb read in place) ----
                    pvs = [pvpool.tile([P, TT, HD + 1], F32, tag=f"pv{h2}",
                                       name=f"pv_{pr}_{h2}")
                           for h2 in range(2)]
                    for gi in range(KT // 2):
                        a = 2 * gi
                        for h2 in range(2):
                            h_idx = 2 * pr + h2
                            rows = slice(h2 * HD, h2 * HD + HD)
                            stp = stpool.tile([P, 1024], F32, tag="st",
                                              name=f"st_{pr}_{a}_{h2}")
                            for j in range(2):
                                kt = a + j
                                nc.tensor.matmul(
                                    stp[:, j * 512:(j + 1) * 512],
                                    kT_sb[rows, kt * P:(kt + 1) * P],
                                    qT_t[rows, pr, :], start=True, stop=True)
                            ptt = atp.tile([P, 2, QT], BF16, tag="pt",
                                           name=f"pt_{pr}_{a}_{h2}")
                            nc.scalar.activation(
                                ptt[:],
                                stp[:].rearrange("p (c n) -> p c n", n=512),
                                AF.Exp, scale=0.125)
                            vsl = v_aug[:, :, :].rearrange(
                                "p t (h c) -> p t h c", c=HD + 1)[:, :, h_idx, :]
                            for j in range(2):
                                kt = a + j
                                for qt in range(TT):
                                    nc.tensor.matmul(
                                        pvs[h2][:, qt, :],
                                        ptt[:, j, qt * P:(qt + 1) * P],
                                        vsl[:, kt, :],
                                        start=(kt == 0), stop=(kt == KT - 1))
                    for h2 in range(2):
                        rcp = atd.tile([P, TT], F32, tag="rcp",
                                       name=f"rcp_{pr}_{h2}")
                        nc.vector.reciprocal(rcp[:], pvs[h2][:, :, HD])
                        nc.vector.tensor_tensor(
                            aqAll[:, pr, :, h2 * HD:(h2 + 1) * HD],
                            pvs[h2][:, :, 0:HD],
                            rcp[:, :, None].to_broadcast([P, TT, HD]),
                            ALU.mult)

            _att_es.close()
            _ffn_es = ExitStack()
            fw = _ffn_es.enter_context(tc.tile_pool(name="fw", bufs=1))

            # ---- transpose attn out [q,hd] -> [hd,q] for the out proj ----
            outSB = pers.tile([P, NPAIR, QT], BF16, tag="tok3")
            with tc.tile_pool(name="tqs", bufs=4, space="PSUM") as tqs:
                for pr in range(NPAIR):
                    for qt in range(TT):
                        tp = tqs.tile([P, P], BF16, tag="tq",
                                      name=f"tq_{pr}_{qt}")
                        nc.tensor.transpose(tp[:], aqAll[:, pr, qt, :], ident[:])
                        nc.vector.tensor_copy(
                            outSB[:, pr, qt * P:(qt + 1) * P], tp[:])

            # ================= Output proj + residual + LN1 ===========
            # Two tt-halves with 4 PSUM banks each: LN1 + hT transposes of
            # half A overlap half B's matmuls.
            h_t = pers.tile([P, TT, D], BF16, tag="tok2")
            hT_t = pers.tile([P, DC, QT], BF16, tag="tok1")
            with tc.tile_pool(name="lnp3", bufs=4) as lnp3, \
                 tc.tile_pool(name="ops", bufs=1, space="PSUM") as ops, \
                 tc.tile_pool(name="tps", bufs=2, space="PSUM") as tps:
                for half in range(2):
                    tts = (2 * half, 2 * half + 1)
                    pso = [ops.tile([P, 512], F32, tag=f"ao{i}",
                                    name=f"ao_{half}_{i}") for i in range(4)]
                    for pr in range(NPAIR):
                        for i, tt in enumerate(tts):
                            for oc in range(2):
                                nc.tensor.matmul(
                                    pso[i * 2 + oc],
                                    outSB[:, pr, tt * P:(tt + 1) * P],
                                    wo_t[:, pr, oc * 512:(oc + 1) * 512],
                                    start=(pr == 0), stop=(pr == NPAIR - 1))
                    for i, tt in enumerate(tts):
                        for oc in range(2):
                            nc.vector.tensor_tensor(
                                h_t[:, tt, oc * 512:(oc + 1) * 512],
                                pso[i * 2 + oc],
                                xq_s[:, tt, oc * 512:(oc + 1) * 512], ALU.add)
                    _layernorm_multi(
                        nc, lnp3,
                        [h_t[:, tt, :] for tt in tts],
                        [h_t[:, tt, :] for tt in tts],
                        g1b_t, be1b_t, eps_t, zero_t, affine)
                    for i, tt in enumerate(tts):
                        for dc in range(DC):
                            pst = tps.tile([P, P], BF16, tag="tp",
                                           name=f"tp_{tt}_{dc}")
                            nc.tensor.transpose(
                                pst[:], h_t[:, tt, dc * P:(dc + 1) * P], ident[:])
                            nc.vector.tensor_copy(
                                hT_t[:, dc, tt * P:(tt + 1) * P], pst[:])

            # ================= FFN + LN2 ==============================
            with tc.tile_pool(name="ft", bufs=4) as ft, \
                 tc.tile_pool(name="lnp4", bufs=4) as lnp4:
                ff1 = pers.tile([P, FC, QT], BF16, tag="blobA")
                with tc.tile_pool(name="f1s", bufs=4, space="PSUM") as f1s:
                    for fc4 in range(FC // 4):
                        w1_t = w1_ts[fc4]
                        for f in range(4):
                            fc = 4 * fc4 + f
                            ps = f1s.tile([P, 512], F32, tag="mm")
                            for dc in range(DC):
                                nc.tensor.matmul(ps[:], w1_t[:, f, dc, :],
                                                 hT_t[:, dc, :],
                                                 start=(dc == 0), stop=(dc == DC - 1))
                            # fused bias + relu
                            nc.vector.tensor_scalar(ff1[:, fc, :], ps[:],
                                                    b1_t[:, fc:fc + 1], 0.0,
                                                    ALU.add, ALU.max)
                        if fc4 + 2 < FC // 4:
                            load_w1(fc4 + 2, (nc.sync, nc.scalar))

                b2b_t = fw.tile([P, D], F32, tag="b2b")
                nc.scalar.dma_start(b2b_t[:], b2b[:])
                # fold the fc2 bias into the residual while ff1 runs
                for tt in range(TT):
                    nc.vector.tensor_tensor(h_t[:, tt, :], h_t[:, tt, :],
                                            b2b_t[:], ALU.add)
                if affine:
                    g2b_t = fw.tile([P, D], F32, tag="g2b")
                    be2b_t = fw.tile([P, D], F32, tag="be2b")
                    nc.scalar.dma_start(g2b_t[:], g2b[:])
                    nc.scalar.dma_start(be2b_t[:], be2b[:])
                else:
                    g2b_t = be2b_t = None
                # FFN2 in a 3+1 tt split, each part with its own full w2
                # stream: part A's (3-row) epilogue hides under part B's
                # matmuls, so only one LayerNorm chain is exposed as tail.
                with tc.tile_pool(name="f2s", bufs=1, space="PSUM") as f2s:
                    for hf, tts in enumerate(((0, 1, 2), (3,))):
                        pss = [f2s.tile([P, 512], F32, tag=f"ff2_{hf}_{i}",
                                        name=f"ff2_{hf}_{i}")
                               for i in range(2 * len(tts))]
                        for fc4 in range(FC // 4):
                            w2_t = fp.tile([P, 4, D], BF16, tag="wstream",
                                           name=f"w2_{hf}_{fc4}")
                            gate(w2_t[0:1, :, 0], 4, epoch2)
                            nc.sync.dma_start(
                                w2_t[:, 0:2, :], w2[4 * fc4:4 * fc4 + 2]
                                .rearrange("f p m -> p f m"))
                            nc.scalar.dma_start(
                                w2_t[:, 2:4, :], w2[4 * fc4 + 2:4 * fc4 + 4]
                                .rearrange("f p m -> p f m"))
                            for f in range(4):
                                fc = 4 * fc4 + f
                                for i, tt in enumerate(tts):
                                    for oc in range(2):
                                        nc.tensor.matmul(
                                            pss[i * 2 + oc],
                                            ff1[:, fc, tt * P:(tt + 1) * P],
                                            w2_t[:, f, oc * 512:(oc + 1) * 512],
                                            start=(fc == 0), stop=(fc == FC - 1))
                        t2s = [ft.tile([P, D], F32, tag="t2", name=f"t2_{tt}")
                               for tt in tts]
                        last = hf == 1
                        for i, tt in enumerate(tts):
                            for oc in range(2):
                                nc.vector.tensor_tensor(
                                    t2s[i][:, oc * 512:(oc + 1) * 512],
                                    pss[i * 2 + oc],
                                    h_t[:, tt, oc * 512:(oc + 1) * 512], ALU.add)
                        y_r = y.rearrange("(t p) d -> p t d", p=P)
                        if last:
                            # per-half final normalize + store so the y DMA
                            # overlaps the second half's normalize
                            def fin(i, sl):
                                nc.sync.dma_start(y_r[:, tts[i], sl],
                                                  t2s[i][:, sl])
                            _layernorm_multi(nc, lnp4, [t[:] for t in t2s],
                                             [t[:] for t in t2s],
                                             g2b_t, be2b_t, eps_t, zero_t,
                                             affine, on_final=fin)
                        else:
                            _layernorm_multi(nc, lnp4, [t[:] for t in t2s],
                                             [t[:] for t in t2s],
                                             g2b_t, be2b_t, eps_t, zero_t,
                                             affine)
                            for i, tt in enumerate(tts):
                                nc.sync.dma_start(y_r[:, tt, :], t2s[i][:])
            _ffn_es.close()
            _pre.close()

    nc.compile()
    return nc


def _layernorm_multi(nc, pool, dsts, srcs, g_t, be_t, eps_t, zero_t, affine,
                     on_final=None):
    """dst = (src - mean)/sqrt(var + eps) [* g + be], row-wise over 1024,
    for several tiles with the per-step emission interleaved so the serial
    latency of one chain hides under its siblings.

    var = E[x^2] - mu^2 (safe here: |mu| << rms). The mean-reduce (DVE) and
    square+sum (ACT, accum_out) run concurrently; one Newton step refines
    rsqrt. c doubles as the square scratch before holding (src - mu).
    """
    n = len(dsts)
    t = {}
    for i in range(n):
        t[i] = {
            "mu": pool.tile([P, 1], F32, tag="ln_mu", name=f"ln_mu{i}"),
            "c": pool.tile([P, D], F32, tag="ln_c", name=f"ln_c{i}"),
            "ss": pool.tile([P, 1], F32, tag="ln_ss", name=f"ln_ss{i}"),
            "vv": pool.tile([P, 1], F32, tag="ln_v", name=f"ln_v{i}"),
            "m2": pool.tile([P, 1], F32, tag="ln_m2", name=f"ln_m2{i}"),
            "s": pool.tile([P, 1], F32, tag="ln_s", name=f"ln_s{i}"),
            "r": pool.tile([P, 1], F32, tag="ln_r", name=f"ln_r{i}"),
            "tt": pool.tile([P, 1], F32, tag="ln_t", name=f"ln_t{i}"),
        }
    for i in range(n):
        nc.vector.tensor_reduce(t[i]["mu"][:], srcs[i], mybir.AxisListType.X,
                                ALU.add)
        nc.scalar.activation(t[i]["c"][:], srcs[i], AF.Square,
                             accum_out=t[i]["ss"][:])
    for i in range(n):
        nc.vector.tensor_scalar_mul(t[i]["mu"][:], t[i]["mu"][:], 1.0 / D)
        nc.vector.tensor_scalar(t[i]["vv"][:], t[i]["ss"][:], 1.0 / D, EPS,
                                ALU.mult, ALU.add)
        nc.vector.tensor_tensor(t[i]["m2"][:], t[i]["mu"][:], t[i]["mu"][:],
                                ALU.mult)
    for i in range(n):
        nc.vector.tensor_tensor(t[i]["vv"][:], t[i]["vv"][:], t[i]["m2"][:],
                                ALU.subtract)
        nc.scalar.activation(t[i]["s"][:], t[i]["vv"][:], AF.Sqrt,
                             bias=zero_t[:])
        nc.vector.reciprocal(t[i]["r"][:], t[i]["s"][:])
    for i in range(n):
        if on_final is None:
            # fused (src - mu) * r in one DVE pass
            nc.vector.tensor_scalar(dsts[i], srcs[i], t[i]["mu"][:],
                                    t[i]["r"][:], ALU.subtract, ALU.mult)
            if affine:
                nc.vector.tensor_tensor(dsts[i], dsts[i], g_t[:], ALU.mult)
                nc.vector.tensor_tensor(dsts[i], dsts[i], be_t[:], ALU.add)
        else:
            for oc in range(2):
                sl = slice(oc * (D // 2), (oc + 1) * (D // 2))
                nc.vector.tensor_scalar(dsts[i][:, sl], srcs[i][:, sl],
                                        t[i]["mu"][:], t[i]["r"][:],
                                        ALU.subtract, ALU.mult)
                if affine:
                    nc.vector.tensor_tensor(dsts[i][:, sl], dsts[i][:, sl],
                                            g_t[:, sl], ALU.mult)
                    nc.vector.tensor_tensor(dsts[i][:, sl], dsts[i][:, sl],
                                            be_t[:, sl], ALU.add)
                on_final(i, sl)


def make_in_maps(x, w_qkv, b_qkv, w_o, b_o, g1, be1, w1, b1, w2, b2, g2, be2):
    f = np.float32
    x = np.asarray(x, f)
    w_qkv = np.asarray(w_qkv, f)
    b_qkv = np.asarray(b_qkv, f)
    bc = lambda v: np.ascontiguousarray(
        np.broadcast_to(np.asarray(v, f).reshape(1, D), (P, D)))
    shared = {
        "wq": np.ascontiguousarray(
            w_qkv[:, :D].reshape(DC, P, NPAIR, P).transpose(2, 0, 1, 3)).astype(BF),
        "wk": np.ascontiguousarray(
            w_qkv[:, D:2 * D].reshape(DC, P, NPAIR, P).transpose(2, 0, 1, 3)).astype(BF),
        "wv": np.ascontiguousarray(
            w_qkv[:, 2 * D:].reshape(DC, P, 2, 512).transpose(2, 0, 1, 3)).astype(BF),
        "wo": np.ascontiguousarray(np.asarray(w_o, f).reshape(DC, P, D)).astype(BF),
        "w1": np.ascontiguousarray(
            np.asarray(w1, f).reshape(DC, P, FC, P).transpose(2, 0, 1, 3)).astype(BF),
        "w2": np.ascontiguousarray(np.asarray(w2, f).reshape(FC, P, D)).astype(BF),
        "bq": np.ascontiguousarray(b_qkv[:D].reshape(NPAIR, P).T),
        "bk": np.ascontiguousarray(b_qkv[D:2 * D].reshape(NPAIR, P).T),
        "b1": np.ascontiguousarray(np.asarray(b1, f).reshape(FC, P).T),
        "bvb": bc(b_qkv[2 * D:]), "b2b": bc(b2),
        "g1b": bc(g1), "be1b": bc(be1), "g2b": bc(g2), "be2b": bc(be2),
        "vones": np.ones((P, KT), BF),
    }
    in_maps = []
    for c in range(8):
        n, qi = divmod(c, 4)
        xTn = np.ascontiguousarray(x[n].T)
        # own tokens first; softmax over keys is order-invariant
        order = np.r_[qi * QT:(qi + 1) * QT, 0:qi * QT, (qi + 1) * QT:L]
        m = dict(shared)
        m["xT"] = np.ascontiguousarray(xTn[:, order]).astype(BF)
        m["xq"] = np.ascontiguousarray(x[n, qi * QT:(qi + 1) * QT, :]
                                 + np.asarray(b_o, f).reshape(1, D))
        in_maps.append(m)
    return in_maps


def get_nc(affine=True):
    if affine not in _CACHED_NC:
        _CACHED_NC[affine] = _build_nc(affine)
    return _CACHED_NC[affine]


def kernel(**inputs):
    in_maps = make_in_maps(**inputs)
    affine = not (np.all(np.asarray(inputs["g1"]) == 1)
                  and np.all(np.asarray(inputs["be1"]) == 0)
                  and np.all(np.asarray(inputs["g2"]) == 1)
                  and np.all(np.asarray(inputs["be2"]) == 0))
    nc = get_nc(affine)
    # The axon-proxied NRT occasionally reports a transient
    # NRT_EXEC_UNIT_UNRECOVERABLE on a cold first dispatch; a plain retry
    # has always succeeded with bit-identical results, so recover inline.
    last_err = None
    for _ in range(3):
        try:
            res = run_bass_kernel_spmd(nc, in_maps, list(range(8))).results
            break
        except Exception as e:  # noqa: BLE001
            last_err = e
    else:
        raise last_err
    y = np.empty((NB, L, D), np.float32)
    for c in range(8):
        n, qi = divmod(c, 4)
        y[n, qi * QT:(qi + 1) * QT] = res[c]["y"]
    return y


if __name__ == "__main__":
    rng = np.random.default_rng(0)
    demo = {
        "x": rng.standard_normal((NB, L, D)).astype(np.float32),
        "w_qkv": rng.standard_normal((D, 3 * D)).astype(np.float32) * 0.03,
        "b_qkv": rng.standard_normal(3 * D).astype(np.float32) * 0.03,
        "w_o": rng.standard_normal((D, D)).astype(np.float32) * 0.03,
        "b_o": rng.standard_normal(D).astype(np.float32) * 0.03,
        "g1": np.ones(D, np.float32), "be1": np.zeros(D, np.float32),
        "w1": rng.standard_normal((D, FF)).astype(np.float32) * 0.03,
        "b1": rng.standard_normal(FF).astype(np.float32) * 0.03,
        "w2": rng.standard_normal((FF, D)).astype(np.float32) * 0.015,
        "b2": rng.standard_normal(D).astype(np.float32) * 0.015,
        "g2": np.ones(D, np.float32), "be2": np.zeros(D, np.float32),
    }
    out = kernel(**demo)
    print("kernel output:", out.shape, out.dtype, np.abs(out).mean())
